# revision 5
# baseline (speedup 1.0000x reference)
"""CenterNet-style decode for Trainium2, batch-parallel over 8 NeuronCores.

kernel(heat[16,80,128,128], wh, reg, K=100) -> [16,100,6] f32, bit-exact vs
the jax reference (ties broken by lowest flat index, as jax top_k).

Scores are k/2^24 uniform in (0,1); anything reaching the global top-100 is
>= 1-2^-8 w.h.p., so scores are mapped EXACTLY and monotonically into u16 via
relu((x - (1-2^-8)) * 2^24) on the otherwise-idle Activation engine (Sterbenz
subtraction + power-of-2 scale keep it exact; below-range values clamp to 0
and are guarded).  The whole NMS max tree + peak mask then runs in u16 at the
DVE 2x rate, h-split between DVE and the GpSimd/Pool engine.  Per-class top-8
via u16 max8/max_index; 13 max/max_index/match_replace rounds extract the
top-104 of the 640-candidate union with exact jax tie semantics; winner f32
scores are reconstructed bit-exactly as (u16 + (2^24-2^16)) * 2^-24.  Winner
metadata (spatial idx, wh, reg) is fetched with per-partition-offset indirect
DMAs in a [winner=partition] column layout.  Guards (flag output): any class
whose 8th-best could displace the 100th winner, or a top-100 value at the u16
clamp boundary.  All partition-reshaping data movement bounces through DRAM
scratch (SBUF->SBUF partition-reshape descriptors fail to load here).
"""

import sys

sys.path.insert(0, "/opt/trn_rl_repo")

import numpy as np

import bass_rust
import concourse.bass as bass
import concourse.tile as tile
from concourse import mybir
from concourse.vector_clock import ScopedClock

B, C, H, W = 16, 80, 128, 128
HW = H * W
K = 100
NCORES = 8
BPC = B // NCORES
KPAD = 104
NU = C * 8
F32 = mybir.dt.float32
U16 = mybir.dt.uint16
F16 = mybir.dt.float16
U32 = mybir.dt.uint32
ALU = mybir.AluOpType
AF = mybir.ActivationFunctionType

# u16 codes restricted to [0, 0x7C00): finite positive f16 bit patterns,
# so the Pool engine (no u16 integer ops) can run the same passes on
# f16-bitcast views with identical bit results.
X0INT = 2**24 - 0x7C00  # 16745472; x0 = X0INT/2^24
SCALE = float(2**24)
HS = 93  # DVE/Pool h-split for max/eq passes (DVE 0.52 ns/col vs Pool 1.389)
HSM = HS  # eq/mult splits must match: DVE u16 mask is 1, Pool f16 mask is 1.0


def _split_excess_waits(nc):
    """This walrus build accepts at most ONE sync wait per instruction.
    Hoist excess waits onto same-engine NoOps inserted just before."""
    for fn in nc.m.functions:
        for bb in fn.blocks:
            new_insts = []
            for inst in bb.instructions:
                si = inst.sync_info
                waits = list(si.on_wait) if (si is not None and si.on_wait) else []
                if len(waits) > 1:
                    si.on_wait = waits[:1]
                    for w in waits[1:]:
                        nop = mybir.InstNoOp(
                            name=nc.get_next_instruction_name(),
                            ins=[],
                            outs=[],
                            hint="waitsplit",
                        )
                        nop.engine = inst.engine
                        nop.sync_info = bass_rust.SyncInfo(on_wait=[w], on_update=[])
                        nc.register_instruction(nop, overwrite=True)
                        new_insts.append(nop)
                new_insts.append(inst)
            bb.instructions[:] = new_insts


def _patched_drain_and_barrier(self, tick_clock, wait_clock):
    nc = self.nc
    drain_inst = nc.sync.drain()
    wait_clock.add_sem_waits(
        drain_inst.ins, ScopedClock({None: tick_clock.global_clock})
    )
    si = drain_inst.ins.sync_info
    waits = list(si.on_wait or []) if si is not None else []
    if waits:
        si.on_wait = []
        for i, w in enumerate(waits):
            n = nc.sync.nop(hint=f"waitsplit{i}", nofuse=True)
            n.ins.sync_info = bass_rust.SyncInfo(on_wait=[w], on_update=[])
    nc.all_engine_barrier()
    assert self.sems is not None
    popped = nc._tile_sem_poison_stack.pop()
    assert popped is self._sem_poison
    nc.clear_and_free_semaphores(list(self.sems.allocated().values()))
    nc.all_engine_barrier()
    _split_excess_waits(nc)


tile.TileContext._drain_and_barrier = _patched_drain_and_barrier


def build_program():
    nc = bass.Bass("TRN2", target_bir_lowering=False, debug=False)

    heat = nc.dram_tensor("heat", [BPC, C, H, W], F32, kind="ExternalInput").ap()
    wh = nc.dram_tensor("wh", [BPC, 2, H, W], F32, kind="ExternalInput").ap()
    reg = nc.dram_tensor("reg", [BPC, 2, H, W], F32, kind="ExternalInput").ap()
    out = nc.dram_tensor("out", [BPC, K, 6], F32, kind="ExternalOutput").ap()
    flags = nc.dram_tensor("flags", [BPC, 2], F32, kind="ExternalOutput").ap()
    scr = {
        "fl_v": nc.dram_tensor("fl_v", [BPC, NU], U16).ap(),
        "fl_i": nc.dram_tensor("fl_i", [BPC, NU], U32).ap(),
        "fl_g": nc.dram_tensor("fl_g", [BPC, C], U16).ap(),
        "xig": nc.dram_tensor("xig_scr", [BPC, KPAD], U32).ap(),
        "sco": nc.dram_tensor("sco_scr", [BPC, KPAD], U16).ap(),
    }

    with tile.TileContext(nc) as tc:
        build_tile_kernel(tc, heat, wh, reg, out, flags, scr)
    return nc


def build_tile_kernel(tc, heat, wh, reg, out, flags, scr):
    from contextlib import ExitStack

    nc = tc.nc
    ctx = ExitStack()
    with ctx:
        big = ctx.enter_context(tc.tile_pool(name="big", bufs=1))
        ld = ctx.enter_context(tc.tile_pool(name="ld", bufs=2))
        sp = ctx.enter_context(tc.tile_pool(name="small", bufs=1))

        bias = sp.tile([C, 1], F32, tag="bias")
        nc.vector.memset(bias[:], float(-X0INT))

        bufA = big.tile([C, HW], U16, tag="bufA")  # u16 heat (per-batch scratch)
        bufT = big.tile([C, HW], U16, tag="bufT")
        bufV = big.tile([C, HW], U16, tag="bufV")
        Z = [
            big.tile([C, HW], U16, tag=f"z{b}", name=f"zbuf{b}")
            for b in range(BPC)
        ]

        heat2 = heat.rearrange("b c h w -> b c (h w)")

        V8, I8 = [], []
        for b in range(BPC):
            # ---- load f32 + convert to u16 on Activation, in 4 chunks ----
            cw = HW // 4
            for kd in range(4):
                xs = ld.tile([C, cw], F32)
                nc.sync.dma_start(xs[:], heat2[b, :, kd * cw : (kd + 1) * cw])
                nc.scalar.activation(
                    bufA[:, kd * cw : (kd + 1) * cw],
                    xs[:],
                    AF.Relu,
                    bias=bias[:],
                    scale=SCALE,
                )
            a3 = bufA[:].rearrange("c (h w) -> c h w", w=W)
            t3 = bufT[:].rearrange("c (h w) -> c h w", w=W)
            v3 = bufV[:].rearrange("c (h w) -> c h w", w=W)
            z3 = Z[b][:].rearrange("c (h w) -> c h w", w=W)

            # ---- pass 1: t[h] = max(a[h], a[h+1]), h in [0,127) ----------
            nc.vector.tensor_tensor(
                out=t3[:, 0 : H - 1], in0=a3[:, 0 : H - 1], in1=a3[:, 1:H],
                op=ALU.max,
            )
            nc.vector.tensor_copy(out=t3[:, H - 1 : H], in_=a3[:, H - 1 : H])
            # ---- pass 2: V[h] = max(t[h], a[h-1]), h in [1,128) ----------
            nc.vector.tensor_tensor(
                out=v3[:, 1:H], in0=t3[:, 1:H], in1=a3[:, 0 : H - 1], op=ALU.max
            )
            nc.vector.tensor_copy(out=v3[:, 0:1], in_=t3[:, 0:1])
            # ---- pass 3: u[w] = max(V[w], V[w+1]), w in [0,127); u=bufT --
            nc.vector.tensor_tensor(
                out=t3[:, :, 0 : W - 1],
                in0=v3[:, :, 0 : W - 1],
                in1=v3[:, :, 1:W],
                op=ALU.max,
            )
            nc.vector.tensor_copy(
                out=t3[:, :, W - 1 : W], in_=v3[:, :, W - 1 : W]
            )
            # ---- pass 4: m[w] = max(u[w], V[w-1]), w in [1,128); m=Z[b] --
            nc.vector.tensor_tensor(
                out=z3[:, :, 1:W],
                in0=t3[:, :, 1:W],
                in1=v3[:, :, 0 : W - 1],
                op=ALU.max,
            )
            nc.vector.tensor_copy(out=z3[:, :, 0:1], in_=t3[:, :, 0:1])
            # ---- pass 5: Z = (Z == a), in place --------------------------
            nc.vector.tensor_tensor(
                out=z3[:], in0=z3[:], in1=a3[:], op=ALU.is_equal
            )
            # ---- pass 6: Z = Z * a, in place -----------------------------
            nc.vector.tensor_tensor(
                out=z3[:], in0=z3[:], in1=a3[:], op=ALU.mult
            )

            # ---- per-class top-8 values + indices ------------------------
            vb = sp.tile([C, 8], U16, tag=f"v{b}")
            ib = sp.tile([C, 8], U32, tag=f"i{b}")
            nc.vector.max(out=vb[:], in_=Z[b][:])
            nc.vector.max_index(out=ib[:], in_max=vb[:], in_values=Z[b][:])
            V8.append(vb)
            I8.append(ib)
            # flatten candidates via DRAM (SBUF->SBUF partition reshapes
            # don't load on this runtime)
            nc.sync.dma_start(
                scr["fl_v"][b].rearrange("(c k) -> c k", k=8), vb[:]
            )
            nc.sync.dma_start(
                scr["fl_i"][b].rearrange("(c k) -> c k", k=8), ib[:]
            )
            nc.sync.dma_start(
                scr["fl_g"][b].rearrange("(c k) -> c k", k=1), vb[:, 7:8]
            )

        uv = sp.tile([BPC, NU], U16, tag="uv")
        g8 = sp.tile([BPC, C], U16, tag="g8")
        nc.sync.dma_start(uv[:], scr["fl_v"][:, :])
        nc.sync.dma_start(g8[:], scr["fl_g"][:, :])

        # ---- extraction: top-104, ties by (value desc, position asc) ----
        S = sp.tile([BPC, KPAD], U16, tag="scores")
        XI = sp.tile([BPC, KPAD], U32, tag="xi")
        for j in range(13):
            sj = S[:, 8 * j : 8 * j + 8]
            nc.vector.max(out=sj, in_=uv[:])
            nc.vector.max_index(
                out=XI[:, 8 * j : 8 * j + 8], in_max=sj, in_values=uv[:]
            )
            if j < 12:
                nc.vector.match_replace(
                    out=uv[:], in_to_replace=sj, in_values=uv[:], imm_value=0.0
                )

        # ---- guards ------------------------------------------------------
        gmax = sp.tile([BPC, 1], U16, tag="gmax")
        nc.vector.tensor_reduce(
            out=gmax[:], in_=g8[:], axis=mybir.AxisListType.X, op=ALU.max
        )
        flg = sp.tile([BPC, 2], U16, tag="flg")
        nc.vector.tensor_tensor(
            out=flg[:, 0:1], in0=gmax[:], in1=S[:, K - 1 : K], op=ALU.is_ge
        )
        nc.vector.tensor_scalar(
            out=flg[:, 1:2], in0=S[:, K - 1 : K], scalar1=0, scalar2=None,
            op0=ALU.is_equal,
        )
        flg_f = sp.tile([BPC, 2], F32, tag="flgf")
        nc.vector.tensor_copy(out=flg_f[:], in_=flg[:])
        nc.sync.dma_start(flags[:, :], flg_f[:])

        # ---- winner positions within the 640-union, to DRAM for the tail
        nc.sync.dma_start(scr["xig"][:, :], XI[:])
        nc.sync.dma_start(scr["sco"][:, :], S[:])

        # ---- per-batch column-layout tail: winner = partition ------------
        fl_i_flat = scr["fl_i"].rearrange("(o b) n -> o (b n)", o=1)
        wh_flat = wh.rearrange("b c h w -> (b c) (h w)")
        reg_flat = reg.rearrange("b c h w -> (b c) (h w)")
        for b in range(BPC):
            xcol = sp.tile([KPAD, 1], U32, tag=f"xcol{b}")
            nc.sync.dma_start(
                xcol[:], scr["xig"][b, :].rearrange("(k o) -> k o", o=1)
            )
            scol = sp.tile([KPAD, 1], U16, tag=f"scol{b}")
            nc.sync.dma_start(
                scol[:], scr["sco"][b, :].rearrange("(k o) -> k o", o=1)
            )
            # exact f32 score: (u16 + X0INT) * 2^-24
            s_f0 = sp.tile([KPAD, 1], F32, tag=f"sf0{b}")
            nc.vector.tensor_copy(out=s_f0[:], in_=scol[:])
            s_f = sp.tile([KPAD, 1], F32, tag=f"sf{b}")
            nc.vector.tensor_scalar(
                out=s_f[:], in0=s_f0[:], scalar1=float(X0INT),
                scalar2=float(2.0**-24), op0=ALU.add, op1=ALU.mult,
            )
            # class = pos//8 ; global union offset for the gather = pos + b*NU
            cls_u = sp.tile([KPAD, 1], U32, tag=f"clsu{b}")
            nc.vector.tensor_scalar(
                out=cls_u[:], in0=xcol[:], scalar1=3, scalar2=None,
                op0=ALU.logical_shift_right,
            )
            cls_f = sp.tile([KPAD, 1], F32, tag=f"clsf{b}")
            nc.vector.tensor_copy(out=cls_f[:], in_=cls_u[:])
            bcNU = sp.tile([KPAD, 1], U32, tag=f"bcNU{b}")
            nc.vector.memset(bcNU[:], b * NU)
            nc.vector.tensor_tensor(
                out=xcol[:], in0=xcol[:], in1=bcNU[:], op=ALU.add
            )
            # spatial index: one gather, per-partition offset, run of 1
            s_u = sp.tile([KPAD, 1], U32, tag=f"su{b}")
            nc.gpsimd.indirect_dma_start(
                out=s_u[:],
                out_offset=None,
                in_=fl_i_flat,
                in_offset=bass.IndirectOffsetOnAxis(ap=xcol[:], axis=1),
            )
            ys_u = sp.tile([KPAD, 1], U32, tag=f"ysu{b}")
            xs_u = sp.tile([KPAD, 1], U32, tag=f"xsu{b}")
            nc.vector.tensor_scalar(
                out=ys_u[:], in0=s_u[:], scalar1=7, scalar2=None,
                op0=ALU.logical_shift_right,
            )
            nc.vector.tensor_scalar(
                out=xs_u[:], in0=s_u[:], scalar1=127, scalar2=None,
                op0=ALU.bitwise_and,
            )
            ys_f = sp.tile([KPAD, 1], F32, tag=f"ysf{b}")
            xs_f = sp.tile([KPAD, 1], F32, tag=f"xsf{b}")
            nc.vector.tensor_copy(out=ys_f[:], in_=ys_u[:])
            nc.vector.tensor_copy(out=xs_f[:], in_=xs_u[:])
            # wh/reg: 4 independent gathers at offsets b*2HW + {0,HW} + s
            wrg = sp.tile([KPAD, 4], F32, tag=f"wrg{b}")
            off0 = sp.tile([KPAD, 1], U32, tag=f"off0{b}")
            off1 = sp.tile([KPAD, 1], U32, tag=f"off1{b}")
            nc.vector.tensor_scalar(
                out=off0[:], in0=s_u[:], scalar1=b * 2 * HW, scalar2=None,
                op0=ALU.add,
            )
            nc.vector.tensor_scalar(
                out=off1[:], in0=s_u[:], scalar1=b * 2 * HW + HW, scalar2=None,
                op0=ALU.add,
            )
            for comp, srct, offt in (
                (0, wh_flat, off0),
                (1, wh_flat, off1),
                (2, reg_flat, off0),
                (3, reg_flat, off1),
            ):
                nc.gpsimd.indirect_dma_start(
                    out=wrg[:, comp : comp + 1],
                    out_offset=None,
                    in_=srct,
                    in_offset=bass.IndirectOffsetOnAxis(ap=offt[:], axis=1),
                )
            # assemble [K, 6] = x1 y1 x2 y2 score class
            kk = slice(0, K)
            xc = sp.tile([KPAD, 1], F32, tag=f"xc{b}")
            yc = sp.tile([KPAD, 1], F32, tag=f"yc{b}")
            h0t = sp.tile([KPAD, 1], F32, tag=f"h0t{b}")
            h1t = sp.tile([KPAD, 1], F32, tag=f"h1t{b}")
            nc.vector.tensor_tensor(
                out=xc[:], in0=xs_f[:], in1=wrg[:, 2:3], op=ALU.add
            )
            nc.vector.tensor_tensor(
                out=yc[:], in0=ys_f[:], in1=wrg[:, 3:4], op=ALU.add
            )
            nc.vector.tensor_scalar_mul(h0t[:], wrg[:, 0:1], 0.5)
            nc.vector.tensor_scalar_mul(h1t[:], wrg[:, 1:2], 0.5)
            ob = sp.tile([KPAD, 6], F32, tag=f"ob{b}")
            nc.vector.tensor_tensor(
                out=ob[:, 0:1], in0=xc[:], in1=h0t[:], op=ALU.subtract
            )
            nc.vector.tensor_tensor(
                out=ob[:, 1:2], in0=yc[:], in1=h1t[:], op=ALU.subtract
            )
            nc.vector.tensor_tensor(out=ob[:, 2:3], in0=xc[:], in1=h0t[:], op=ALU.add)
            nc.vector.tensor_tensor(out=ob[:, 3:4], in0=yc[:], in1=h1t[:], op=ALU.add)
            nc.vector.tensor_copy(out=ob[:, 4:5], in_=s_f[:])
            nc.vector.tensor_copy(out=ob[:, 5:6], in_=cls_f[:])
            nc.sync.dma_start(out[b], ob[kk, :])


_NC_CACHE = {}


def _get_program():
    if "nc" not in _NC_CACHE:
        _NC_CACHE["nc"] = build_program()
    return _NC_CACHE["nc"]


def kernel(heat, wh, reg, K):
    assert int(K) == 100
    heat = np.ascontiguousarray(np.asarray(heat, dtype=np.float32))
    wh = np.ascontiguousarray(np.asarray(wh, dtype=np.float32))
    reg = np.ascontiguousarray(np.asarray(reg, dtype=np.float32))
    assert heat.shape == (B, C, H, W)

    nc = _get_program()
    in_maps = []
    for i in range(NCORES):
        sl = slice(i * BPC, (i + 1) * BPC)
        in_maps.append(
            {
                "heat": np.ascontiguousarray(heat[sl]),
                "wh": np.ascontiguousarray(wh[sl]),
                "reg": np.ascontiguousarray(reg[sl]),
            }
        )
    from concourse.bass_utils import run_bass_kernel_spmd

    res = run_bass_kernel_spmd(nc, in_maps, list(range(NCORES)))
    outs = []
    for i in range(NCORES):
        r = res.results[i]
        if np.any(r["flags"] != 0.0):
            raise RuntimeError(f"top-k guard tripped on core {i}")
        outs.append(r["out"])
    return np.concatenate(outs, axis=0)
